# revision 17
# baseline (speedup 1.0000x reference)
"""CacheFuser Trainium2 Bass kernel.

Sharding: layer-parallel — 8 layers -> 8 NeuronCores, one layer per core.
Each core fuses its layer's K and V caches for all B*S tokens.

Math (per layer, per cache c in {k, v}, tokens t, hidden h):
    H_n   = ReLU((X_n @ w1) * e_n/4 + b1 * e_n/4)        n = 0..3 sharers
    G     = sum_n H_n                    (edge-weighted, post-ReLU aggregate)
    P     = R @ fw1a + G @ (w2 @ fw1b)   (aligner 2nd matmul folded into
                                          fusion 1st matmul: w2p precomputed)
    F     = ReLU(P + fb1_eff)            fb1_eff = fb1 + (sum_n e_n/4 * b2) @ fw1b
    D     = F @ fw2 + fb2
    out   = R + sigmoid(alpha/TAU) * D

On-chip dataflow: activations live feature-major ([h, t]); inputs are
loaded natural ([t, h]) fp32, cast to bf16 (matmul operands), transposed
on the TensorEngine via identity matmuls. The final delta is transposed
back to [t, h] and combined with the fp32 residual by a single DVE
scalar_tensor_tensor op reading PSUM.
"""
import sys
import os

sys.path.insert(0, "/opt/trn_rl_repo")

import numpy as np
import ml_dtypes

L, N, B, S, H = 8, 4, 2, 4096, 256
T = B * S
TAU = 0.5
TS = 512           # tokens per tile iteration
NT = T // TS       # 16 iterations

_CACHE = {}


def _build_program():
    import concourse.bacc as bacc
    import concourse.mybir as mybir
    from concourse.tile import TileContext
    from concourse.masks import make_identity

    F32 = mybir.dt.float32
    BF16 = mybir.dt.bfloat16
    Relu = mybir.ActivationFunctionType.Relu
    Identity = mybir.ActivationFunctionType.Identity
    MULT = mybir.AluOpType.mult
    ADD = mybir.AluOpType.add

    nc = bacc.Bacc()

    # ---- DRAM parameters (per-core slices; fp32 unless noted) ----
    rk_d = nc.declare_dram_parameter("rk", [T, H], F32, isOutput=False)
    rv_d = nc.declare_dram_parameter("rv", [T, H], F32, isOutput=False)
    sk_d = nc.declare_dram_parameter("sk", [N, T, H], F32, isOutput=False)
    sv_d = nc.declare_dram_parameter("sv", [N, T, H], F32, isOutput=False)
    w_d = {}
    for c in ("k", "v"):
        for nm in ("w1", "w2p", "fw1a", "fw2"):
            w_d[c, nm] = nc.declare_dram_parameter(f"{nm}{c}", [H, H], BF16, isOutput=False)
        w_d[c, "b1s"] = nc.declare_dram_parameter(f"b1s{c}", [128, 2, N], F32, isOutput=False)
        w_d[c, "fb1"] = nc.declare_dram_parameter(f"fb1{c}", [128, 2], F32, isOutput=False)
        w_d[c, "fb2"] = nc.declare_dram_parameter(f"fb2{c}", [128, 2], F32, isOutput=False)
    esc_d = nc.declare_dram_parameter("esc", [128, N], F32, isOutput=False)
    gate_d = nc.declare_dram_parameter("gate", [128, 1], F32, isOutput=False)
    out_d = nc.declare_dram_parameter("out", [2, T, H], F32, isOutput=True)

    r_d = {"k": rk_d, "v": rv_d}
    s_d = {"k": sk_d, "v": sv_d}

    with TileContext(nc) as tc:
        with tc.tile_pool(name="const", bufs=1) as cpool, \
             tc.tile_pool(name="sb", bufs=2) as pool, \
             tc.tile_pool(name="big", bufs=2) as bpool, \
             tc.tile_pool(name="psmm", bufs=4, space="PSUM") as mmp, \
             tc.tile_pool(name="pstr", bufs=4, space="PSUM") as trp:

            ident = cpool.tile([128, 128], BF16)
            make_identity(nc, ident)

            # constants / weights
            wt = {}
            for c in ("k", "v"):
                for nm in ("w1", "w2p", "fw1a", "fw2"):
                    t_ = cpool.tile([128, 2, H], BF16, tag=f"{nm}{c}")
                    nc.scalar.dma_start(out=t_, in_=w_d[c, nm].rearrange("(kc p) h -> p kc h", p=128))
                    wt[c, nm] = t_
                for nm, shp in (("b1s", [128, 2, N]), ("fb1", [128, 2]), ("fb2", [128, 2])):
                    t_ = cpool.tile(shp, F32, tag=f"{nm}{c}")
                    nc.scalar.dma_start(out=t_, in_=w_d[c, nm][...])
                    wt[c, nm] = t_
            esc_t = cpool.tile([128, N], F32)
            nc.scalar.dma_start(out=esc_t, in_=esc_d[...])
            gate_t = cpool.tile([128, 1], F32)
            nc.scalar.dma_start(out=gate_t, in_=gate_d[...])

            CS = ("k", "v")

            def transpose_in(src_bf, tag):
                dst = pool.tile([128, 2, TS], BF16, tag=tag, bufs=3)
                pt = trp.tile([128, 2 * TS], BF16, tag="ps_t")
                for kc in range(2):
                    for o in range(4):
                        nc.tensor.transpose(pt[:, kc * TS + o * 128: kc * TS + (o + 1) * 128],
                                            src_bf[:, o, kc * 128:(kc + 1) * 128], ident)
                nc.any.tensor_copy(out=dst.rearrange("p a b -> p (a b)"), in_=pt)
                return dst

            for it in range(NT):
                tsl = slice(it * TS, (it + 1) * TS)
                st = {c: {} for c in CS}

                # ---- loads (both caches up front for deep prefetch) ----
                for c in CS:
                    rx32 = bpool.tile([128, 4, H], F32, tag=f"rx32{c}")
                    nc.scalar.dma_start(out=rx32, in_=r_d[c][tsl, :].rearrange("(o p) h -> p o h", p=128))
                    st[c]["rx32"] = rx32
                    st[c]["sxb"] = []
                    for n in range(N):
                        # SWDGE cast-load: fp32 DRAM -> bf16 SBUF
                        sb = pool.tile([128, 4, H], BF16, tag=f"sxb{n}{c}", bufs=3)
                        nc.gpsimd.dma_start(out=sb, in_=s_d[c][n, tsl, :].rearrange("(o p) h -> p o h", p=128))
                        st[c]["sxb"].append(sb)
                for c in CS:
                    rxb = pool.tile([128, 4, H], BF16, tag=f"rxb{c}")
                    nc.vector.tensor_copy(out=rxb, in_=st[c]["rx32"])
                    st[c]["rxb"] = rxb

                # ---- transposes + first layer, interleaved across caches ----
                for c in CS:
                    st[c]["sxt"] = [transpose_in(st[c]["sxb"][n], f"sxt{n}{c}") for n in range(N)]
                    rxt = pool.tile([128, 2, TS], BF16, tag=f"rxt{c}", bufs=3)
                    for o in range(4):
                        nc.sync.dma_start(out=rxt[:, :, o * 128:(o + 1) * 128],
                                          in_=st[c]["rxb"][:, o, :], transpose=True)
                    st[c]["rxt"] = rxt

                for c in CS:
                    w1 = wt[c, "w1"]
                    G = pool.tile([128, 2, TS], BF16, tag=f"G{c}")
                    for n in range(N):
                        hn = G if n == 0 else pool.tile([128, 2, TS], BF16, tag=f"hn{c}")
                        for m in range(2):
                            ph = mmp.tile([128, TS], F32, tag="ps_mm")
                            for kc in range(2):
                                nc.tensor.matmul(ph, lhsT=w1[:, kc, m * 128:(m + 1) * 128],
                                                 rhs=st[c]["sxt"][n][:, kc, :],
                                                 start=(kc == 0), stop=(kc == 1))
                            nc.scalar.activation(out=hn[:, m, :], in_=ph, func=Relu,
                                                 bias=wt[c, "b1s"][:, m, n:n + 1],
                                                 scale=esc_t[:, n:n + 1])
                        if n > 0:
                            nc.vector.tensor_add(out=G.rearrange("p a b -> p (a b)"),
                                                 in0=G.rearrange("p a b -> p (a b)"),
                                                 in1=hn.rearrange("p a b -> p (a b)"))
                    st[c]["G"] = G

                # ---- fusion matmuls ----
                for c in CS:
                    fw1a, w2p = wt[c, "fw1a"], wt[c, "w2p"]
                    G, rxt = st[c]["G"], st[c]["rxt"]
                    F_t = pool.tile([128, 2, TS], BF16, tag=f"F{c}")
                    for m in range(2):
                        pp = mmp.tile([128, TS], F32, tag="ps_mm")
                        nc.tensor.matmul(pp, lhsT=fw1a[:, 0, m * 128:(m + 1) * 128], rhs=rxt[:, 0, :], start=True, stop=False)
                        nc.tensor.matmul(pp, lhsT=fw1a[:, 1, m * 128:(m + 1) * 128], rhs=rxt[:, 1, :], start=False, stop=False)
                        nc.tensor.matmul(pp, lhsT=w2p[:, 0, m * 128:(m + 1) * 128], rhs=G[:, 0, :], start=False, stop=False)
                        nc.tensor.matmul(pp, lhsT=w2p[:, 1, m * 128:(m + 1) * 128], rhs=G[:, 1, :], start=False, stop=True)
                        nc.scalar.activation(out=F_t[:, m, :], in_=pp, func=Relu,
                                             bias=wt[c, "fb1"][:, m:m + 1])
                    st[c]["F"] = F_t

                for c in CS:
                    fw2 = wt[c, "fw2"]
                    D_t = pool.tile([128, 2, TS], BF16, tag=f"D{c}")
                    for m in range(2):
                        pd = mmp.tile([128, TS], F32, tag="ps_mm")
                        for kc in range(2):
                            nc.tensor.matmul(pd, lhsT=fw2[:, kc, m * 128:(m + 1) * 128],
                                             rhs=st[c]["F"][:, kc, :],
                                             start=(kc == 0), stop=(kc == 1))
                        nc.scalar.activation(out=D_t[:, m, :], in_=pd, func=Identity,
                                             bias=wt[c, "fb2"][:, m:m + 1])
                    st[c]["D"] = D_t

                # ---- delta transpose via xbar + gated residual + store ----
                for c in CS:
                    D_t, rx32 = st[c]["D"], st[c]["rx32"]
                    # dtt[p, m, o, d_lo] = D_t[m-chunk d, o*128+p]
                    dtt = pool.tile([128, 2, 4, 128], BF16, tag=f"dtt{c}")
                    for m in range(2):
                        nc.sync.dma_start(out=dtt[:, m, :, :], in_=D_t[:, m, :], transpose=True)
                    o32 = bpool.tile([128, 4, H], F32, tag=f"o32{c}")
                    for o in range(4):
                        nc.vector.scalar_tensor_tensor(out=o32[:, o, :].rearrange("p (a b) -> p a b", a=2),
                                                       in0=dtt[:, :, o, :],
                                                       scalar=gate_t[:, 0:1],
                                                       in1=rx32[:, o, :].rearrange("p (a b) -> p a b", a=2),
                                                       op0=MULT, op1=ADD)
                    nc.scalar.dma_start(out=out_d[0 if c == "k" else 1, tsl, :]
                                        .rearrange("(o p) h -> p o h", p=128),
                                        in_=o32)

    nc.finalize()
    return nc


def _sigmoid(x):
    return 1.0 / (1.0 + np.exp(-x))


def _part_major(vec):
    """[H] bias vector -> [128, 2] partition-major layout (chunk m on free axis)."""
    return np.ascontiguousarray(vec.reshape(2, 128).T.astype(np.float32))


def _prep_in_maps(inputs):
    bf = ml_dtypes.bfloat16
    in_maps = []
    for l in range(L):
        e = np.asarray(inputs["edge_weights"][l], np.float32)
        esc = e / N                                     # [4]
        gate = _sigmoid(float(inputs["alpha"][l]) / TAU)
        m = {
            "rk": np.ascontiguousarray(inputs["receiver_k"][l].reshape(T, H), np.float32),
            "rv": np.ascontiguousarray(inputs["receiver_v"][l].reshape(T, H), np.float32),
            "sk": np.ascontiguousarray(inputs["sharer_k"][l].reshape(N, T, H), np.float32),
            "sv": np.ascontiguousarray(inputs["sharer_v"][l].reshape(N, T, H), np.float32),
            "esc": np.ascontiguousarray(np.broadcast_to(esc[None, :], (128, N)), np.float32),
            "gate": np.full((128, 1), gate, np.float32),
        }
        for c, (w1, b1, w2, b2, fw1, fb1, fw2, fb2) in {
            "k": (inputs["ak_w1"][l], inputs["ak_b1"][l], inputs["ak_w2"][l], inputs["ak_b2"][l],
                  inputs["fk_w1"][l], inputs["fk_b1"][l], inputs["fk_w2"][l], inputs["fk_b2"][l]),
            "v": (inputs["av_w1"][l], inputs["av_b1"][l], inputs["av_w2"][l], inputs["av_b2"][l],
                  inputs["fv_w1"][l], inputs["fv_b1"][l], inputs["fv_w2"][l], inputs["fv_b2"][l]),
        }.items():
            w1 = np.asarray(w1, np.float32)
            fw1 = np.asarray(fw1, np.float32)
            w2 = np.asarray(w2, np.float32)
            fw1a, fw1b = fw1[:H], fw1[H:]
            w2p = w2 @ fw1b                              # folded aligner matmul
            fb1_eff = np.asarray(fb1, np.float32) + (esc.sum() * np.asarray(b2, np.float32)) @ fw1b
            b1s = np.asarray(b1, np.float32)[None, :] * esc[:, None]   # [N, H]
            b1s_pm = np.stack([_part_major(b1s[n]) for n in range(N)], axis=2)  # [128,2,N]
            m[f"w1{c}"] = w1.astype(bf)
            m[f"w2p{c}"] = w2p.astype(bf)
            m[f"fw1a{c}"] = np.ascontiguousarray(fw1a).astype(bf)
            m[f"fw2{c}"] = np.asarray(fw2, np.float32).astype(bf)
            m[f"b1s{c}"] = np.ascontiguousarray(b1s_pm)
            m[f"fb1{c}"] = _part_major(fb1_eff)
            m[f"fb2{c}"] = _part_major(np.asarray(fb2, np.float32))
        in_maps.append(m)
    return in_maps


def _run(inputs, trace=False):
    from concourse.bass_utils import run_bass_kernel_spmd

    if "nc" not in _CACHE:
        _CACHE["nc"] = _build_program()
    nc = _CACHE["nc"]
    in_maps = _prep_in_maps(inputs)
    res = run_bass_kernel_spmd(nc, in_maps, list(range(L)), trace=trace)
    outs = [np.asarray(res.results[l]["out"]) for l in range(L)]     # [2, T, H] each
    full = np.stack(outs, axis=1)                                    # [2, L, T, H]
    return full.reshape(2, L, B, S, H).astype(np.float32), res


def kernel(**inputs):
    out, _ = _run(inputs, trace=False)
    return out


def kernel_traced(**inputs):
    """Like kernel() but also returns the profiled hardware exec time (ns)."""
    out, res = _run(inputs, trace=True)
    return out, res.exec_time_ns


# revision 18
# speedup vs baseline: 1.5945x; 1.5945x over previous
"""CacheFuser Trainium2 Bass kernel.

Sharding: layer-parallel — 8 layers -> 8 NeuronCores, one layer per core.
Each core fuses its layer's K and V caches for all B*S tokens.

Math (per layer, per cache c in {k, v}, tokens t, hidden h):
    H_n   = ReLU((X_n @ w1) * e_n/4 + b1 * e_n/4)        n = 0..3 sharers
    G     = sum_n H_n                    (edge-weighted, post-ReLU aggregate)
    P     = R @ fw1a + G @ (w2 @ fw1b)   (aligner 2nd matmul folded into
                                          fusion 1st matmul: w2p precomputed)
    F     = ReLU(P + fb1_eff)            fb1_eff = fb1 + (sum_n e_n/4 * b2) @ fw1b
    D     = F @ fw2 + fb2
    out   = R + sigmoid(alpha/TAU) * D

On-chip dataflow: activations live feature-major ([h, t]); inputs are
loaded natural ([t, h]) fp32, cast to bf16 (matmul operands), transposed
on the TensorEngine via identity matmuls. The final delta is transposed
back to [t, h] and combined with the fp32 residual by a single DVE
scalar_tensor_tensor op reading PSUM.
"""
import sys
import os

sys.path.insert(0, "/opt/trn_rl_repo")

import numpy as np
import ml_dtypes

L, N, B, S, H = 8, 4, 2, 4096, 256
T = B * S
TAU = 0.5
TS = 512           # tokens per tile iteration
NT = T // TS       # 16 iterations

_CACHE = {}


def _build_program(zero_bias=False):
    import concourse.bacc as bacc
    import concourse.mybir as mybir
    from concourse.tile import TileContext
    from concourse.masks import make_identity

    F32 = mybir.dt.float32
    BF16 = mybir.dt.bfloat16
    Relu = mybir.ActivationFunctionType.Relu
    Identity = mybir.ActivationFunctionType.Identity
    MULT = mybir.AluOpType.mult
    ADD = mybir.AluOpType.add
    MAXOP = mybir.AluOpType.max

    nc = bacc.Bacc()

    # ---- DRAM parameters (per-core slices; fp32 unless noted) ----
    rk_d = nc.declare_dram_parameter("rk", [T, H], F32, isOutput=False)
    rv_d = nc.declare_dram_parameter("rv", [T, H], F32, isOutput=False)
    sk_d = nc.declare_dram_parameter("sk", [N, T, H], F32, isOutput=False)
    sv_d = nc.declare_dram_parameter("sv", [N, T, H], F32, isOutput=False)
    w_d = {}
    for c in ("k", "v"):
        for nm in ("w1", "w2p", "fw1a", "fw2"):
            w_d[c, nm] = nc.declare_dram_parameter(f"{nm}{c}", [H, H], BF16, isOutput=False)
        w_d[c, "b1s"] = nc.declare_dram_parameter(f"b1s{c}", [128, 2, N], F32, isOutput=False)
        w_d[c, "fb1"] = nc.declare_dram_parameter(f"fb1{c}", [128, 2], F32, isOutput=False)
        w_d[c, "fb2"] = nc.declare_dram_parameter(f"fb2{c}", [128, 2], F32, isOutput=False)
    esc_d = nc.declare_dram_parameter("esc", [128, N], F32, isOutput=False)
    gate_d = nc.declare_dram_parameter("gate", [128, 1], F32, isOutput=False)
    out_d = nc.declare_dram_parameter("out", [2, T, H], F32, isOutput=True)

    r_d = {"k": rk_d, "v": rv_d}
    s_d = {"k": sk_d, "v": sv_d}

    with TileContext(nc) as tc:
        with tc.tile_pool(name="const", bufs=1) as cpool, \
             tc.tile_pool(name="sb", bufs=2) as pool, \
             tc.tile_pool(name="big", bufs=2) as bpool, \
             tc.tile_pool(name="psmm", bufs=4, space="PSUM") as mmp, \
             tc.tile_pool(name="pstr", bufs=4, space="PSUM") as trp:

            ident = cpool.tile([128, 128], BF16)
            make_identity(nc, ident)

            # constants / weights
            wt = {}
            for c in ("k", "v"):
                for nm in ("w1", "w2p", "fw1a", "fw2"):
                    t_ = cpool.tile([128, 2, H], BF16, tag=f"{nm}{c}")
                    nc.scalar.dma_start(out=t_, in_=w_d[c, nm].rearrange("(kc p) h -> p kc h", p=128))
                    wt[c, nm] = t_
                for nm, shp in (("b1s", [128, 2, N]), ("fb1", [128, 2]), ("fb2", [128, 2])):
                    t_ = cpool.tile(shp, F32, tag=f"{nm}{c}")
                    nc.scalar.dma_start(out=t_, in_=w_d[c, nm][...])
                    wt[c, nm] = t_
            esc_t = cpool.tile([128, N], F32)
            nc.scalar.dma_start(out=esc_t, in_=esc_d[...])
            gate_t = cpool.tile([128, 1], F32)
            nc.scalar.dma_start(out=gate_t, in_=gate_d[...])

            CS = ("k", "v")

            def transpose_in(src_bf, tag):
                dst = pool.tile([128, 2, TS], BF16, tag=tag, bufs=3)
                pt = trp.tile([128, 2 * TS], BF16, tag="ps_t")
                for kc in range(2):
                    for o in range(4):
                        nc.tensor.transpose(pt[:, kc * TS + o * 128: kc * TS + (o + 1) * 128],
                                            src_bf[:, o, kc * 128:(kc + 1) * 128], ident)
                nc.scalar.copy(out=dst.rearrange("p a b -> p (a b)"), in_=pt)
                return dst

            for it in range(NT):
                tsl = slice(it * TS, (it + 1) * TS)
                st = {c: {} for c in CS}

                # ---- loads (both caches up front for deep prefetch) ----
                for c in CS:
                    rx32 = bpool.tile([128, 4, H], F32, tag=f"rx32{c}")
                    nc.scalar.dma_start(out=rx32, in_=r_d[c][tsl, :].rearrange("(o p) h -> p o h", p=128))
                    st[c]["rx32"] = rx32
                    st[c]["sxb"] = []
                    for n in range(N):
                        # SWDGE cast-load: fp32 DRAM -> bf16 SBUF
                        sb = pool.tile([128, 4, H], BF16, tag=f"sxb{n}{c}", bufs=3)
                        nc.gpsimd.dma_start(out=sb, in_=s_d[c][n, tsl, :].rearrange("(o p) h -> p o h", p=128))
                        st[c]["sxb"].append(sb)
                for c in CS:
                    rxb = pool.tile([128, 4, H], BF16, tag=f"rxb{c}")
                    nc.vector.tensor_copy(out=rxb, in_=st[c]["rx32"])
                    st[c]["rxb"] = rxb

                # ---- transposes + first layer, interleaved across caches ----
                for c in CS:
                    st[c]["sxt"] = [transpose_in(st[c]["sxb"][n], f"sxt{n}{c}") for n in range(N)]
                    st[c]["rxt"] = transpose_in(st[c]["rxb"], f"rxt{c}")

                for c in CS:
                    w1 = wt[c, "w1"]
                    G = pool.tile([128, 2, TS], BF16, tag=f"G{c}")
                    for n in range(N):
                        hn = G if n == 0 else pool.tile([128, 2, TS], BF16, tag=f"hn{c}")
                        for m in range(2):
                            ph = mmp.tile([128, TS], F32, tag="ps_mm")
                            for kc in range(2):
                                nc.tensor.matmul(ph, lhsT=w1[:, kc, m * 128:(m + 1) * 128],
                                                 rhs=st[c]["sxt"][n][:, kc, :],
                                                 start=(kc == 0), stop=(kc == 1))
                            if zero_bias and n % 2 == 1:
                                nc.vector.tensor_scalar(out=hn[:, m, :], in0=ph,
                                                        scalar1=esc_t[:, n:n + 1], scalar2=0.0,
                                                        op0=MULT, op1=MAXOP)
                            else:
                                nc.scalar.activation(out=hn[:, m, :], in_=ph, func=Relu,
                                                     bias=wt[c, "b1s"][:, m, n:n + 1],
                                                     scale=esc_t[:, n:n + 1])
                        if n > 0:
                            nc.vector.tensor_add(out=G.rearrange("p a b -> p (a b)"),
                                                 in0=G.rearrange("p a b -> p (a b)"),
                                                 in1=hn.rearrange("p a b -> p (a b)"))
                    st[c]["G"] = G

                # ---- fusion matmuls ----
                for c in CS:
                    fw1a, w2p = wt[c, "fw1a"], wt[c, "w2p"]
                    G, rxt = st[c]["G"], st[c]["rxt"]
                    F_t = pool.tile([128, 2, TS], BF16, tag=f"F{c}")
                    for m in range(2):
                        pp = mmp.tile([128, TS], F32, tag="ps_mm")
                        nc.tensor.matmul(pp, lhsT=fw1a[:, 0, m * 128:(m + 1) * 128], rhs=rxt[:, 0, :], start=True, stop=False)
                        nc.tensor.matmul(pp, lhsT=fw1a[:, 1, m * 128:(m + 1) * 128], rhs=rxt[:, 1, :], start=False, stop=False)
                        nc.tensor.matmul(pp, lhsT=w2p[:, 0, m * 128:(m + 1) * 128], rhs=G[:, 0, :], start=False, stop=False)
                        nc.tensor.matmul(pp, lhsT=w2p[:, 1, m * 128:(m + 1) * 128], rhs=G[:, 1, :], start=False, stop=True)
                        nc.scalar.activation(out=F_t[:, m, :], in_=pp, func=Relu,
                                             bias=wt[c, "fb1"][:, m:m + 1])
                    st[c]["F"] = F_t

                for c in CS:
                    fw2 = wt[c, "fw2"]
                    D_t = pool.tile([128, 2, TS], BF16, tag=f"D{c}")
                    for m in range(2):
                        pd = mmp.tile([128, TS], F32, tag="ps_mm")
                        for kc in range(2):
                            nc.tensor.matmul(pd, lhsT=fw2[:, kc, m * 128:(m + 1) * 128],
                                             rhs=st[c]["F"][:, kc, :],
                                             start=(kc == 0), stop=(kc == 1))
                        nc.scalar.activation(out=D_t[:, m, :], in_=pd, func=Identity,
                                             bias=wt[c, "fb2"][:, m:m + 1])
                    st[c]["D"] = D_t

                # ---- delta transpose + gated residual + store ----
                for c in CS:
                    D_t, rx32 = st[c]["D"], st[c]["rx32"]
                    o32 = bpool.tile([128, 4, H], F32, tag=f"o32{c}")
                    for op_ in range(2):
                        pdt = trp.tile([128, 2 * TS], BF16, tag="ps_t")
                        for oo in range(2):
                            o = op_ * 2 + oo
                            for m in range(2):
                                nc.tensor.transpose(pdt[:, oo * H + m * 128: oo * H + (m + 1) * 128],
                                                    D_t[:, m, o * 128:(o + 1) * 128], ident)
                        for oo in range(2):
                            o = op_ * 2 + oo
                            nc.vector.scalar_tensor_tensor(out=o32[:, o, :],
                                                           in0=pdt[:, oo * H:(oo + 1) * H],
                                                           scalar=gate_t[:, 0:1],
                                                           in1=rx32[:, o, :],
                                                           op0=MULT, op1=ADD)
                    nc.scalar.dma_start(out=out_d[0 if c == "k" else 1, tsl, :]
                                        .rearrange("(o p) h -> p o h", p=128),
                                        in_=o32)

    nc.finalize()
    return nc


def _sigmoid(x):
    return 1.0 / (1.0 + np.exp(-x))


def _part_major(vec):
    """[H] bias vector -> [128, 2] partition-major layout (chunk m on free axis)."""
    return np.ascontiguousarray(vec.reshape(2, 128).T.astype(np.float32))


def _prep_in_maps(inputs):
    bf = ml_dtypes.bfloat16
    in_maps = []
    zero_bias = True
    for l in range(L):
        e = np.asarray(inputs["edge_weights"][l], np.float32)
        esc = e / N                                     # [4]
        gate = _sigmoid(float(inputs["alpha"][l]) / TAU)
        m = {
            "rk": np.ascontiguousarray(inputs["receiver_k"][l].reshape(T, H), np.float32),
            "rv": np.ascontiguousarray(inputs["receiver_v"][l].reshape(T, H), np.float32),
            "sk": np.ascontiguousarray(inputs["sharer_k"][l].reshape(N, T, H), np.float32),
            "sv": np.ascontiguousarray(inputs["sharer_v"][l].reshape(N, T, H), np.float32),
            "esc": np.ascontiguousarray(np.broadcast_to(esc[None, :], (128, N)), np.float32),
            "gate": np.full((128, 1), gate, np.float32),
        }
        for c, (w1, b1, w2, b2, fw1, fb1, fw2, fb2) in {
            "k": (inputs["ak_w1"][l], inputs["ak_b1"][l], inputs["ak_w2"][l], inputs["ak_b2"][l],
                  inputs["fk_w1"][l], inputs["fk_b1"][l], inputs["fk_w2"][l], inputs["fk_b2"][l]),
            "v": (inputs["av_w1"][l], inputs["av_b1"][l], inputs["av_w2"][l], inputs["av_b2"][l],
                  inputs["fv_w1"][l], inputs["fv_b1"][l], inputs["fv_w2"][l], inputs["fv_b2"][l]),
        }.items():
            w1 = np.asarray(w1, np.float32)
            fw1 = np.asarray(fw1, np.float32)
            w2 = np.asarray(w2, np.float32)
            fw1a, fw1b = fw1[:H], fw1[H:]
            w2p = w2 @ fw1b                              # folded aligner matmul
            fb1_eff = np.asarray(fb1, np.float32) + (esc.sum() * np.asarray(b2, np.float32)) @ fw1b
            b1s = np.asarray(b1, np.float32)[None, :] * esc[:, None]   # [N, H]
            b1s_pm = np.stack([_part_major(b1s[n]) for n in range(N)], axis=2)  # [128,2,N]
            m[f"w1{c}"] = w1.astype(bf)
            m[f"w2p{c}"] = w2p.astype(bf)
            m[f"fw1a{c}"] = np.ascontiguousarray(fw1a).astype(bf)
            m[f"fw2{c}"] = np.asarray(fw2, np.float32).astype(bf)
            m[f"b1s{c}"] = np.ascontiguousarray(b1s_pm)
            m[f"fb1{c}"] = _part_major(fb1_eff)
            m[f"fb2{c}"] = _part_major(np.asarray(fb2, np.float32))
            if m[f"b1s{c}"].any() or m[f"fb1{c}"].any() or m[f"fb2{c}"].any():
                zero_bias = False
        in_maps.append(m)
    return in_maps, zero_bias


def _run(inputs, trace=False):
    from concourse.bass_utils import run_bass_kernel_spmd

    in_maps, zero_bias = _prep_in_maps(inputs)
    key = ("nc", zero_bias)
    if key not in _CACHE:
        _CACHE[key] = _build_program(zero_bias)
    nc = _CACHE[key]
    res = run_bass_kernel_spmd(nc, in_maps, list(range(L)), trace=trace)
    outs = [np.asarray(res.results[l]["out"]) for l in range(L)]     # [2, T, H] each
    full = np.stack(outs, axis=1)                                    # [2, L, T, H]
    return full.reshape(2, L, B, S, H).astype(np.float32), res


def kernel(**inputs):
    out, _ = _run(inputs, trace=False)
    return out


def kernel_traced(**inputs):
    """Like kernel() but also returns the profiled hardware exec time (ns)."""
    out, res = _run(inputs, trace=True)
    return out, res.exec_time_ns


# revision 19
# speedup vs baseline: 1.8252x; 1.1447x over previous
"""CacheFuser Trainium2 Bass kernel.

Sharding: layer-parallel — 8 layers -> 8 NeuronCores, one layer per core.
Each core fuses its layer's K and V caches for all B*S tokens.

Math (per layer, per cache c in {k, v}, tokens t, hidden h):
    H_n   = ReLU((X_n @ w1) * e_n/4 + b1 * e_n/4)        n = 0..3 sharers
    G     = sum_n H_n                    (edge-weighted, post-ReLU aggregate)
    P     = R @ fw1a + G @ (w2 @ fw1b)   (aligner 2nd matmul folded into
                                          fusion 1st matmul: w2p precomputed)
    F     = ReLU(P + fb1_eff)            fb1_eff = fb1 + (sum_n e_n/4 * b2) @ fw1b
    D     = F @ fw2 + fb2
    out   = R + sigmoid(alpha/TAU) * D

On-chip dataflow: activations live feature-major ([h, t]); inputs are
loaded natural ([t, h]) fp32, cast to bf16 (matmul operands), transposed
on the TensorEngine via identity matmuls. The final delta is transposed
back to [t, h] and combined with the fp32 residual by a single DVE
scalar_tensor_tensor op reading PSUM.
"""
import sys
import os

sys.path.insert(0, "/opt/trn_rl_repo")

import numpy as np
import ml_dtypes

L, N, B, S, H = 8, 4, 2, 4096, 256
T = B * S
TAU = 0.5
TS = 512           # tokens per tile iteration
NT = T // TS       # 16 iterations

_CACHE = {}


def _build_program(zero_bias=False):
    import concourse.bacc as bacc
    import concourse.mybir as mybir
    from concourse.tile import TileContext
    from concourse.masks import make_identity

    F32 = mybir.dt.float32
    BF16 = mybir.dt.bfloat16
    Relu = mybir.ActivationFunctionType.Relu
    Identity = mybir.ActivationFunctionType.Identity
    MULT = mybir.AluOpType.mult
    ADD = mybir.AluOpType.add
    MAXOP = mybir.AluOpType.max

    nc = bacc.Bacc()

    # ---- DRAM parameters (per-core slices; fp32 unless noted) ----
    rk_d = nc.declare_dram_parameter("rk", [T, H], F32, isOutput=False)
    rv_d = nc.declare_dram_parameter("rv", [T, H], F32, isOutput=False)
    sk_d = nc.declare_dram_parameter("sk", [N, T, H], F32, isOutput=False)
    sv_d = nc.declare_dram_parameter("sv", [N, T, H], F32, isOutput=False)
    w_d = {}
    for c in ("k", "v"):
        for nm in ("w1", "w2p", "fw1a", "fw2"):
            w_d[c, nm] = nc.declare_dram_parameter(f"{nm}{c}", [H, H], BF16, isOutput=False)
        w_d[c, "b1s"] = nc.declare_dram_parameter(f"b1s{c}", [128, 2, N], F32, isOutput=False)
        w_d[c, "fb1"] = nc.declare_dram_parameter(f"fb1{c}", [128, 2], F32, isOutput=False)
        w_d[c, "fb2"] = nc.declare_dram_parameter(f"fb2{c}", [128, 2], F32, isOutput=False)
    esc_d = nc.declare_dram_parameter("esc", [128, N], F32, isOutput=False)
    gate_d = nc.declare_dram_parameter("gate", [128, 1], F32, isOutput=False)
    out_d = nc.declare_dram_parameter("out", [2, T, H], F32, isOutput=True)

    r_d = {"k": rk_d, "v": rv_d}
    s_d = {"k": sk_d, "v": sv_d}

    with TileContext(nc) as tc:
        with tc.tile_pool(name="const", bufs=1) as cpool, \
             tc.tile_pool(name="sb", bufs=2) as pool, \
             tc.tile_pool(name="big", bufs=2) as bpool, \
             tc.tile_pool(name="psmm", bufs=4, space="PSUM") as mmp, \
             tc.tile_pool(name="pstr", bufs=4, space="PSUM") as trp:

            ident = cpool.tile([128, 128], BF16)
            make_identity(nc, ident)

            # constants / weights
            wt = {}
            for c in ("k", "v"):
                for nm in ("w1", "w2p", "fw1a", "fw2"):
                    t_ = cpool.tile([128, 2, H], BF16, tag=f"{nm}{c}")
                    nc.scalar.dma_start(out=t_, in_=w_d[c, nm].rearrange("(kc p) h -> p kc h", p=128))
                    wt[c, nm] = t_
                for nm, shp in (("b1s", [128, 2, N]), ("fb1", [128, 2]), ("fb2", [128, 2])):
                    t_ = cpool.tile(shp, F32, tag=f"{nm}{c}")
                    nc.scalar.dma_start(out=t_, in_=w_d[c, nm][...])
                    wt[c, nm] = t_
            esc_t = cpool.tile([128, N], F32)
            nc.scalar.dma_start(out=esc_t, in_=esc_d[...])
            gate_t = cpool.tile([128, 1], F32)
            nc.scalar.dma_start(out=gate_t, in_=gate_d[...])

            CS = ("k", "v")

            def transpose_in(src_bf, tag):
                dst = pool.tile([128, 2, TS], BF16, tag=tag, bufs=3)
                pt = trp.tile([128, 2 * TS], BF16, tag="ps_t")
                for kc in range(2):
                    for o in range(4):
                        nc.tensor.transpose(pt[:, kc * TS + o * 128: kc * TS + (o + 1) * 128],
                                            src_bf[:, o, kc * 128:(kc + 1) * 128], ident)
                nc.any.tensor_copy(out=dst.rearrange("p a b -> p (a b)"), in_=pt)
                return dst

            for it in range(NT):
                tsl = slice(it * TS, (it + 1) * TS)
                st = {c: {} for c in CS}

                # ---- loads (both caches up front for deep prefetch) ----
                for c in CS:
                    rx32 = bpool.tile([128, 4, H], F32, tag=f"rx32{c}")
                    nc.scalar.dma_start(out=rx32, in_=r_d[c][tsl, :].rearrange("(o p) h -> p o h", p=128))
                    st[c]["rx32"] = rx32
                    st[c]["sxb"] = []
                    for n in range(N):
                        # SWDGE cast-load: fp32 DRAM -> bf16 SBUF
                        sb = pool.tile([128, 4, H], BF16, tag=f"sxb{n}{c}", bufs=3)
                        nc.gpsimd.dma_start(out=sb, in_=s_d[c][n, tsl, :].rearrange("(o p) h -> p o h", p=128))
                        st[c]["sxb"].append(sb)
                for c in CS:
                    rxb = pool.tile([128, 4, H], BF16, tag=f"rxb{c}")
                    nc.vector.tensor_copy(out=rxb, in_=st[c]["rx32"])
                    st[c]["rxb"] = rxb

                # ---- transposes + first layer, interleaved across caches ----
                for c in CS:
                    st[c]["sxt"] = [transpose_in(st[c]["sxb"][n], f"sxt{n}{c}") for n in range(N)]
                    st[c]["rxt"] = transpose_in(st[c]["rxb"], f"rxt{c}")

                for c in CS:
                    w1 = wt[c, "w1"]
                    G = pool.tile([128, 2, TS], BF16, tag=f"G{c}")
                    for n in range(N):
                        hn = G if n == 0 else pool.tile([128, 2, TS], BF16, tag=f"hn{c}")
                        for m in range(2):
                            ph = mmp.tile([128, TS], F32, tag="ps_mm")
                            for kc in range(2):
                                nc.tensor.matmul(ph, lhsT=w1[:, kc, m * 128:(m + 1) * 128],
                                                 rhs=st[c]["sxt"][n][:, kc, :],
                                                 start=(kc == 0), stop=(kc == 1))
                            if zero_bias and n % 2 == 1:
                                nc.vector.tensor_scalar(out=hn[:, m, :], in0=ph,
                                                        scalar1=esc_t[:, n:n + 1], scalar2=0.0,
                                                        op0=MULT, op1=MAXOP)
                            else:
                                nc.scalar.activation(out=hn[:, m, :], in_=ph, func=Relu,
                                                     bias=wt[c, "b1s"][:, m, n:n + 1],
                                                     scale=esc_t[:, n:n + 1])
                        if n > 0:
                            nc.vector.tensor_add(out=G.rearrange("p a b -> p (a b)"),
                                                 in0=G.rearrange("p a b -> p (a b)"),
                                                 in1=hn.rearrange("p a b -> p (a b)"))
                    st[c]["G"] = G

                # ---- fusion matmuls ----
                for c in CS:
                    fw1a, w2p = wt[c, "fw1a"], wt[c, "w2p"]
                    G, rxt = st[c]["G"], st[c]["rxt"]
                    F_t = pool.tile([128, 2, TS], BF16, tag=f"F{c}")
                    for m in range(2):
                        pp = mmp.tile([128, TS], F32, tag="ps_mm")
                        nc.tensor.matmul(pp, lhsT=fw1a[:, 0, m * 128:(m + 1) * 128], rhs=rxt[:, 0, :], start=True, stop=False)
                        nc.tensor.matmul(pp, lhsT=fw1a[:, 1, m * 128:(m + 1) * 128], rhs=rxt[:, 1, :], start=False, stop=False)
                        nc.tensor.matmul(pp, lhsT=w2p[:, 0, m * 128:(m + 1) * 128], rhs=G[:, 0, :], start=False, stop=False)
                        nc.tensor.matmul(pp, lhsT=w2p[:, 1, m * 128:(m + 1) * 128], rhs=G[:, 1, :], start=False, stop=True)
                        nc.scalar.activation(out=F_t[:, m, :], in_=pp, func=Relu,
                                             bias=wt[c, "fb1"][:, m:m + 1])
                    st[c]["F"] = F_t

                for c in CS:
                    fw2 = wt[c, "fw2"]
                    D_t = pool.tile([128, 2, TS], BF16, tag=f"D{c}")
                    for m in range(2):
                        pd = mmp.tile([128, TS], F32, tag="ps_mm")
                        for kc in range(2):
                            nc.tensor.matmul(pd, lhsT=fw2[:, kc, m * 128:(m + 1) * 128],
                                             rhs=st[c]["F"][:, kc, :],
                                             start=(kc == 0), stop=(kc == 1))
                        nc.scalar.activation(out=D_t[:, m, :], in_=pd, func=Identity,
                                             bias=wt[c, "fb2"][:, m:m + 1])
                    st[c]["D"] = D_t

                # ---- delta transpose + gated residual + store ----
                for c in CS:
                    D_t, rx32 = st[c]["D"], st[c]["rx32"]
                    o32 = bpool.tile([128, 4, H], F32, tag=f"o32{c}")
                    for op_ in range(2):
                        pdt = trp.tile([128, 2 * TS], BF16, tag="ps_t")
                        for oo in range(2):
                            o = op_ * 2 + oo
                            for m in range(2):
                                nc.tensor.transpose(pdt[:, oo * H + m * 128: oo * H + (m + 1) * 128],
                                                    D_t[:, m, o * 128:(o + 1) * 128], ident)
                        for oo in range(2):
                            o = op_ * 2 + oo
                            nc.vector.scalar_tensor_tensor(out=o32[:, o, :],
                                                           in0=pdt[:, oo * H:(oo + 1) * H],
                                                           scalar=gate_t[:, 0:1],
                                                           in1=rx32[:, o, :],
                                                           op0=MULT, op1=ADD)
                    nc.scalar.dma_start(out=out_d[0 if c == "k" else 1, tsl, :]
                                        .rearrange("(o p) h -> p o h", p=128),
                                        in_=o32)

    nc.finalize()
    return nc


def _sigmoid(x):
    return 1.0 / (1.0 + np.exp(-x))


def _part_major(vec):
    """[H] bias vector -> [128, 2] partition-major layout (chunk m on free axis)."""
    return np.ascontiguousarray(vec.reshape(2, 128).T.astype(np.float32))


def _prep_in_maps(inputs):
    bf = ml_dtypes.bfloat16
    in_maps = []
    zero_bias = True
    for l in range(L):
        e = np.asarray(inputs["edge_weights"][l], np.float32)
        esc = e / N                                     # [4]
        gate = _sigmoid(float(inputs["alpha"][l]) / TAU)
        m = {
            "rk": np.ascontiguousarray(inputs["receiver_k"][l].reshape(T, H), np.float32),
            "rv": np.ascontiguousarray(inputs["receiver_v"][l].reshape(T, H), np.float32),
            "sk": np.ascontiguousarray(inputs["sharer_k"][l].reshape(N, T, H), np.float32),
            "sv": np.ascontiguousarray(inputs["sharer_v"][l].reshape(N, T, H), np.float32),
            "esc": np.ascontiguousarray(np.broadcast_to(esc[None, :], (128, N)), np.float32),
            "gate": np.full((128, 1), gate, np.float32),
        }
        for c, (w1, b1, w2, b2, fw1, fb1, fw2, fb2) in {
            "k": (inputs["ak_w1"][l], inputs["ak_b1"][l], inputs["ak_w2"][l], inputs["ak_b2"][l],
                  inputs["fk_w1"][l], inputs["fk_b1"][l], inputs["fk_w2"][l], inputs["fk_b2"][l]),
            "v": (inputs["av_w1"][l], inputs["av_b1"][l], inputs["av_w2"][l], inputs["av_b2"][l],
                  inputs["fv_w1"][l], inputs["fv_b1"][l], inputs["fv_w2"][l], inputs["fv_b2"][l]),
        }.items():
            w1 = np.asarray(w1, np.float32)
            fw1 = np.asarray(fw1, np.float32)
            w2 = np.asarray(w2, np.float32)
            fw1a, fw1b = fw1[:H], fw1[H:]
            w2p = w2 @ fw1b                              # folded aligner matmul
            fb1_eff = np.asarray(fb1, np.float32) + (esc.sum() * np.asarray(b2, np.float32)) @ fw1b
            b1s = np.asarray(b1, np.float32)[None, :] * esc[:, None]   # [N, H]
            b1s_pm = np.stack([_part_major(b1s[n]) for n in range(N)], axis=2)  # [128,2,N]
            m[f"w1{c}"] = w1.astype(bf)
            m[f"w2p{c}"] = w2p.astype(bf)
            m[f"fw1a{c}"] = np.ascontiguousarray(fw1a).astype(bf)
            m[f"fw2{c}"] = np.asarray(fw2, np.float32).astype(bf)
            m[f"b1s{c}"] = np.ascontiguousarray(b1s_pm)
            m[f"fb1{c}"] = _part_major(fb1_eff)
            m[f"fb2{c}"] = _part_major(np.asarray(fb2, np.float32))
            if m[f"b1s{c}"].any() or m[f"fb1{c}"].any() or m[f"fb2{c}"].any():
                zero_bias = False
        in_maps.append(m)
    return in_maps, zero_bias


def _run(inputs, trace=False):
    from concourse.bass_utils import run_bass_kernel_spmd

    in_maps, zero_bias = _prep_in_maps(inputs)
    key = ("nc", zero_bias)
    if key not in _CACHE:
        _CACHE[key] = _build_program(zero_bias)
    nc = _CACHE[key]
    res = run_bass_kernel_spmd(nc, in_maps, list(range(L)), trace=trace)
    outs = [np.asarray(res.results[l]["out"]) for l in range(L)]     # [2, T, H] each
    full = np.stack(outs, axis=1)                                    # [2, L, T, H]
    return full.reshape(2, L, B, S, H).astype(np.float32), res


def kernel(**inputs):
    out, _ = _run(inputs, trace=False)
    return out


def kernel_traced(**inputs):
    """Like kernel() but also returns the profiled hardware exec time (ns)."""
    out, res = _run(inputs, trace=True)
    return out, res.exec_time_ns


# revision 21
# speedup vs baseline: 1.8734x; 1.0264x over previous
"""CacheFuser Trainium2 Bass kernel.

Sharding: layer-parallel — 8 layers -> 8 NeuronCores, one layer per core.
Each core fuses its layer's K and V caches for all B*S tokens.

Math (per layer, per cache c in {k, v}, tokens t, hidden h):
    H_n   = ReLU((X_n @ w1) * e_n/4 + b1 * e_n/4)        n = 0..3 sharers
    G     = sum_n H_n                    (edge-weighted, post-ReLU aggregate)
    P     = R @ fw1a + G @ (w2 @ fw1b)   (aligner 2nd matmul folded into
                                          fusion 1st matmul: w2p precomputed)
    F     = ReLU(P + fb1_eff)            fb1_eff = fb1 + (sum_n e_n/4 * b2) @ fw1b
    D     = F @ fw2 + fb2
    out   = R + sigmoid(alpha/TAU) * D

On-chip dataflow: activations live feature-major ([h, t]); inputs are
loaded natural ([t, h]) fp32, cast to bf16 (matmul operands), transposed
on the TensorEngine via identity matmuls. The final delta is transposed
back to [t, h] and combined with the fp32 residual by a single DVE
scalar_tensor_tensor op reading PSUM.
"""
import sys
import os

sys.path.insert(0, "/opt/trn_rl_repo")

import numpy as np
import ml_dtypes

L, N, B, S, H = 8, 4, 2, 4096, 256
T = B * S
TAU = 0.5
TS = 512           # tokens per tile iteration
NT = T // TS       # 16 iterations

_CACHE = {}


def _build_program():
    import concourse.bacc as bacc
    import concourse.mybir as mybir
    from concourse.tile import TileContext
    from concourse.masks import make_identity

    F32 = mybir.dt.float32
    BF16 = mybir.dt.bfloat16
    Relu = mybir.ActivationFunctionType.Relu
    Identity = mybir.ActivationFunctionType.Identity
    MULT = mybir.AluOpType.mult
    ADD = mybir.AluOpType.add

    nc = bacc.Bacc()

    # ---- DRAM parameters (per-core slices; fp32 unless noted) ----
    rk_d = nc.declare_dram_parameter("rk", [T, H], F32, isOutput=False)
    rv_d = nc.declare_dram_parameter("rv", [T, H], F32, isOutput=False)
    sk_d = nc.declare_dram_parameter("sk", [N, T, H], F32, isOutput=False)
    sv_d = nc.declare_dram_parameter("sv", [N, T, H], F32, isOutput=False)
    w_d = {}
    for c in ("k", "v"):
        for nm in ("w1", "w2p", "fw1a", "fw2"):
            w_d[c, nm] = nc.declare_dram_parameter(f"{nm}{c}", [H, H], BF16, isOutput=False)
        w_d[c, "b1s"] = nc.declare_dram_parameter(f"b1s{c}", [128, 2, N], F32, isOutput=False)
        w_d[c, "fb1"] = nc.declare_dram_parameter(f"fb1{c}", [128, 2], F32, isOutput=False)
        w_d[c, "fb2"] = nc.declare_dram_parameter(f"fb2{c}", [128, 2], F32, isOutput=False)
    esc_d = nc.declare_dram_parameter("esc", [128, N], F32, isOutput=False)
    gate_d = nc.declare_dram_parameter("gate", [128, 1], F32, isOutput=False)
    out_d = nc.declare_dram_parameter("out", [2, T, H], F32, isOutput=True)

    r_d = {"k": rk_d, "v": rv_d}
    s_d = {"k": sk_d, "v": sv_d}

    with TileContext(nc) as tc:
        with tc.tile_pool(name="const", bufs=1) as cpool, \
             tc.tile_pool(name="sb", bufs=2) as pool, \
             tc.tile_pool(name="big", bufs=2) as bpool, \
             tc.tile_pool(name="psmm", bufs=4, space="PSUM") as mmp, \
             tc.tile_pool(name="pstr", bufs=4, space="PSUM") as trp:

            ident = cpool.tile([128, 128], BF16)
            make_identity(nc, ident)

            # constants / weights
            wt = {}
            for c in ("k", "v"):
                for nm in ("w1", "w2p", "fw1a", "fw2"):
                    t_ = cpool.tile([128, 2, H], BF16, tag=f"{nm}{c}")
                    nc.scalar.dma_start(out=t_, in_=w_d[c, nm].rearrange("(kc p) h -> p kc h", p=128))
                    wt[c, nm] = t_
                for nm, shp in (("b1s", [128, 2, N]), ("fb1", [128, 2]), ("fb2", [128, 2])):
                    t_ = cpool.tile(shp, F32, tag=f"{nm}{c}")
                    nc.scalar.dma_start(out=t_, in_=w_d[c, nm][...])
                    wt[c, nm] = t_
            esc_t = cpool.tile([128, N], F32)
            nc.scalar.dma_start(out=esc_t, in_=esc_d[...])
            gate_t = cpool.tile([128, 1], F32)
            nc.scalar.dma_start(out=gate_t, in_=gate_d[...])

            CS = ("k", "v")

            def transpose_in(src_bf, tag):
                dst = pool.tile([128, 2, TS], BF16, tag=tag, bufs=3)
                pt = trp.tile([128, 2 * TS], BF16, tag="ps_t")
                for kc in range(2):
                    for o in range(4):
                        nc.tensor.transpose(pt[:, kc * TS + o * 128: kc * TS + (o + 1) * 128],
                                            src_bf[:, o, kc * 128:(kc + 1) * 128], ident)
                nc.any.tensor_copy(out=dst.rearrange("p a b -> p (a b)"), in_=pt)
                return dst

            for it in range(NT):
                tsl = slice(it * TS, (it + 1) * TS)
                st = {c: {} for c in CS}

                # ---- loads (both caches up front for deep prefetch) ----
                for c in CS:
                    rx32 = bpool.tile([128, 4, H], F32, tag=f"rx32{c}")
                    nc.scalar.dma_start(out=rx32, in_=r_d[c][tsl, :].rearrange("(o p) h -> p o h", p=128))
                    st[c]["rx32"] = rx32
                    st[c]["sxb"] = []
                    for n in range(N):
                        # SWDGE cast-load: fp32 DRAM -> bf16 SBUF
                        sb = pool.tile([128, 4, H], BF16, tag=f"sxb{n}{c}", bufs=3)
                        nc.gpsimd.dma_start(out=sb, in_=s_d[c][n, tsl, :].rearrange("(o p) h -> p o h", p=128))
                        st[c]["sxb"].append(sb)
                for c in CS:
                    rxb = pool.tile([128, 4, H], BF16, tag=f"rxb{c}")
                    nc.vector.tensor_copy(out=rxb, in_=st[c]["rx32"])
                    st[c]["rxb"] = rxb

                # ---- transposes + first layer, interleaved across caches ----
                for c in CS:
                    st[c]["sxt"] = [transpose_in(st[c]["sxb"][n], f"sxt{n}{c}") for n in range(N)]
                    st[c]["rxt"] = transpose_in(st[c]["rxb"], f"rxt{c}")

                for c in CS:
                    w1 = wt[c, "w1"]
                    G = pool.tile([128, 2, TS], BF16, tag=f"G{c}")
                    for n in range(N):
                        hn = G if n == 0 else pool.tile([128, 2, TS], BF16, tag=f"hn{c}")
                        for m in range(2):
                            ph = mmp.tile([128, TS], F32, tag="ps_mm")
                            for kc in range(2):
                                nc.tensor.matmul(ph, lhsT=w1[:, kc, m * 128:(m + 1) * 128],
                                                 rhs=st[c]["sxt"][n][:, kc, :],
                                                 start=(kc == 0), stop=(kc == 1))
                            nc.scalar.activation(out=hn[:, m, :], in_=ph, func=Relu,
                                                 bias=wt[c, "b1s"][:, m, n:n + 1],
                                                 scale=esc_t[:, n:n + 1])
                        if n > 0:
                            nc.vector.tensor_add(out=G.rearrange("p a b -> p (a b)"),
                                                 in0=G.rearrange("p a b -> p (a b)"),
                                                 in1=hn.rearrange("p a b -> p (a b)"))
                    st[c]["G"] = G

                # ---- fusion matmuls ----
                for c in CS:
                    fw1a, w2p = wt[c, "fw1a"], wt[c, "w2p"]
                    G, rxt = st[c]["G"], st[c]["rxt"]
                    F_t = pool.tile([128, 2, TS], BF16, tag=f"F{c}")
                    for m in range(2):
                        pp = mmp.tile([128, TS], F32, tag="ps_mm")
                        nc.tensor.matmul(pp, lhsT=fw1a[:, 0, m * 128:(m + 1) * 128], rhs=rxt[:, 0, :], start=True, stop=False)
                        nc.tensor.matmul(pp, lhsT=fw1a[:, 1, m * 128:(m + 1) * 128], rhs=rxt[:, 1, :], start=False, stop=False)
                        nc.tensor.matmul(pp, lhsT=w2p[:, 0, m * 128:(m + 1) * 128], rhs=G[:, 0, :], start=False, stop=False)
                        nc.tensor.matmul(pp, lhsT=w2p[:, 1, m * 128:(m + 1) * 128], rhs=G[:, 1, :], start=False, stop=True)
                        nc.scalar.activation(out=F_t[:, m, :], in_=pp, func=Relu,
                                             bias=wt[c, "fb1"][:, m:m + 1])
                    st[c]["F"] = F_t

                for c in CS:
                    fw2 = wt[c, "fw2"]
                    D_t = pool.tile([128, 2, TS], BF16, tag=f"D{c}")
                    for m in range(2):
                        pd = mmp.tile([128, TS], F32, tag="ps_mm")
                        for kc in range(2):
                            nc.tensor.matmul(pd, lhsT=fw2[:, kc, m * 128:(m + 1) * 128],
                                             rhs=st[c]["F"][:, kc, :],
                                             start=(kc == 0), stop=(kc == 1))
                        nc.vector.tensor_scalar(out=D_t[:, m, :], in0=pd,
                                                scalar1=wt[c, "fb2"][:, m:m + 1],
                                                scalar2=None, op0=ADD)
                    st[c]["D"] = D_t

                # ---- delta transpose + gated residual + store ----
                for c in CS:
                    D_t, rx32 = st[c]["D"], st[c]["rx32"]
                    o32 = bpool.tile([128, 4, H], F32, tag=f"o32{c}")
                    for op_ in range(2):
                        pdt = trp.tile([128, 2 * TS], BF16, tag="ps_t")
                        for oo in range(2):
                            o = op_ * 2 + oo
                            for m in range(2):
                                nc.tensor.transpose(pdt[:, oo * H + m * 128: oo * H + (m + 1) * 128],
                                                    D_t[:, m, o * 128:(o + 1) * 128], ident)
                        for oo in range(2):
                            o = op_ * 2 + oo
                            nc.vector.scalar_tensor_tensor(out=o32[:, o, :],
                                                           in0=pdt[:, oo * H:(oo + 1) * H],
                                                           scalar=gate_t[:, 0:1],
                                                           in1=rx32[:, o, :],
                                                           op0=MULT, op1=ADD)
                    nc.sync.dma_start(out=out_d[0 if c == "k" else 1, tsl, :]
                                        .rearrange("(o p) h -> p o h", p=128),
                                        in_=o32)

    nc.finalize()
    return nc


def _sigmoid(x):
    return 1.0 / (1.0 + np.exp(-x))


def _part_major(vec):
    """[H] bias vector -> [128, 2] partition-major layout (chunk m on free axis)."""
    return np.ascontiguousarray(vec.reshape(2, 128).T.astype(np.float32))


def _prep_in_maps(inputs):
    bf = ml_dtypes.bfloat16
    in_maps = []
    for l in range(L):
        e = np.asarray(inputs["edge_weights"][l], np.float32)
        esc = e / N                                     # [4]
        gate = _sigmoid(float(inputs["alpha"][l]) / TAU)
        m = {
            "rk": np.ascontiguousarray(inputs["receiver_k"][l].reshape(T, H), np.float32),
            "rv": np.ascontiguousarray(inputs["receiver_v"][l].reshape(T, H), np.float32),
            "sk": np.ascontiguousarray(inputs["sharer_k"][l].reshape(N, T, H), np.float32),
            "sv": np.ascontiguousarray(inputs["sharer_v"][l].reshape(N, T, H), np.float32),
            "esc": np.ascontiguousarray(np.broadcast_to(esc[None, :], (128, N)), np.float32),
            "gate": np.full((128, 1), gate, np.float32),
        }
        for c, (w1, b1, w2, b2, fw1, fb1, fw2, fb2) in {
            "k": (inputs["ak_w1"][l], inputs["ak_b1"][l], inputs["ak_w2"][l], inputs["ak_b2"][l],
                  inputs["fk_w1"][l], inputs["fk_b1"][l], inputs["fk_w2"][l], inputs["fk_b2"][l]),
            "v": (inputs["av_w1"][l], inputs["av_b1"][l], inputs["av_w2"][l], inputs["av_b2"][l],
                  inputs["fv_w1"][l], inputs["fv_b1"][l], inputs["fv_w2"][l], inputs["fv_b2"][l]),
        }.items():
            w1 = np.asarray(w1, np.float32)
            fw1 = np.asarray(fw1, np.float32)
            w2 = np.asarray(w2, np.float32)
            fw1a, fw1b = fw1[:H], fw1[H:]
            w2p = w2 @ fw1b                              # folded aligner matmul
            fb1_eff = np.asarray(fb1, np.float32) + (esc.sum() * np.asarray(b2, np.float32)) @ fw1b
            b1s = np.asarray(b1, np.float32)[None, :] * esc[:, None]   # [N, H]
            b1s_pm = np.stack([_part_major(b1s[n]) for n in range(N)], axis=2)  # [128,2,N]
            m[f"w1{c}"] = w1.astype(bf)
            m[f"w2p{c}"] = w2p.astype(bf)
            m[f"fw1a{c}"] = np.ascontiguousarray(fw1a).astype(bf)
            m[f"fw2{c}"] = np.asarray(fw2, np.float32).astype(bf)
            m[f"b1s{c}"] = np.ascontiguousarray(b1s_pm)
            m[f"fb1{c}"] = _part_major(fb1_eff)
            m[f"fb2{c}"] = _part_major(np.asarray(fb2, np.float32))
        in_maps.append(m)
    return in_maps


def _run(inputs, trace=False):
    from concourse.bass_utils import run_bass_kernel_spmd

    if "nc" not in _CACHE:
        _CACHE["nc"] = _build_program()
    nc = _CACHE["nc"]
    in_maps = _prep_in_maps(inputs)
    res = run_bass_kernel_spmd(nc, in_maps, list(range(L)), trace=trace)
    outs = [np.asarray(res.results[l]["out"]) for l in range(L)]     # [2, T, H] each
    full = np.stack(outs, axis=1)                                    # [2, L, T, H]
    return full.reshape(2, L, B, S, H).astype(np.float32), res


def kernel(**inputs):
    out, _ = _run(inputs, trace=False)
    return out


def kernel_traced(**inputs):
    """Like kernel() but also returns the profiled hardware exec time (ns)."""
    out, res = _run(inputs, trace=True)
    return out, res.exec_time_ns
